# revision 5
# baseline (speedup 1.0000x reference)
"""CSPN (convolutional spatial propagation) step on 8 Trainium2 NeuronCores.

Computation (per batch element b, pixel (y, x)):
    out[b,0,y,x] = sum_{t=0..24} w[b,t,2+y,2+x] * src_t[b, y+2-t//5, x+2-t%5]
where src_t = h0 for the center tap (t=12) and hn otherwise, with zero
padding outside the image.

Sharding: B*H = 4*352 = 1408 output rows. Each core gets:
  - chunk A: one 128-row band  (batch c//2, rows 128*(c%2) .. +128)
  - chunk B: half of that batch's remaining 96-row band, split by columns
    (rows 256..352, cols 608*(c%2) .. +608)
so all 8 cores run an identical program on identically-shaped slices.

Device layout: H rows on SBUF partitions, W on the free dimension.  Row
(dy) shifts are pre-resolved by a single DMA that loads 5 row-shifted
copies of the padded source; column (dx) shifts are free-dim offsets.
For 16-bit dtypes a second, one-element-shifted copy of each source row
block keeps every window 4-byte aligned so tensor_tensor runs in the 2x
DVE perf mode.  Per tap group (one dy row = 5 taps): multiply weight
planes by shifted source windows, pairwise-tree the 5 products, and
tree the 5 group results at the end.
"""

import numpy as np

import concourse.bass as bass
import concourse.mybir as mybir
import concourse.tile as tile
from concourse.bass_utils import run_bass_kernel_spmd

K = 5
R = 2
B, H, W = 4, 352, 1216

# chunk name -> (partitions, out width, src block width, slab rows, slab width)
CHUNKS = (
    ("A", 128, 1216, 1220, 132, 1222),
    ("B", 96, 608, 612, 100, 614),
)

N_CORES = 8


def _split_drain_waits(nc):
    """walrus in this container accepts at most one sync-wait per
    instruction; move the extras onto NoOps placed just before it."""
    for bb in nc.main_func.blocks:
        insts = bb.bb.instructions if hasattr(bb, "bb") else bb.instructions
        i = 0
        while i < len(insts):
            ins = insts[i]
            if (
                ins.sync_info
                and ins.sync_info.on_wait
                and len(ins.sync_info.on_wait) > 1
            ):
                extras = ins.sync_info.on_wait[1:]
                ins.sync_info.on_wait = ins.sync_info.on_wait[:1]
                for j, wcond in enumerate(extras):
                    nop = mybir.InstNoOp(
                        name=f"{ins.name}-waitsplit-{j}",
                        ins=[],
                        outs=[],
                        engine=ins.engine,
                        sync_info=mybir.SyncInfo(on_wait=[wcond], on_update=[]),
                    )
                    insts.insert(i, nop)
                    i += 1
            i += 1


def _build_nc(np_dtype, repeat=1, bench=False):
    cdt = mybir.dt.float16 if np_dtype == np.float16 else mybir.dt.float32
    two_byte = np_dtype == np.float16
    npar = 2 if two_byte else 1
    add = mybir.AluOpType.add
    mult = mybir.AluOpType.mult

    nc = bass.Bass()
    dram = {}
    if bench:
        # timing variant: data lives in internal (uninitialized) DRAM so each
        # call ships ~nothing over the wire; tiny external tensors for binding
        dram["_in"] = nc.declare_dram_parameter("_in", [1, 128], cdt, isOutput=False)
        dram["_out"] = nc.declare_dram_parameter("_out", [1, 128], cdt, isOutput=True)
    for nm, P, Wd, WBLK, SR, SW in CHUNKS:
        if bench:
            dram["w" + nm] = nc.dram_tensor("w" + nm, [K * K, P, Wd], cdt)
            dram["src" + nm] = nc.dram_tensor("src" + nm, [SR, SW], cdt)
            dram["h0" + nm] = nc.dram_tensor("h0" + nm, [P, Wd], cdt)
            dram["out" + nm] = nc.dram_tensor("out" + nm, [P, Wd], cdt)
        else:
            dram["w" + nm] = nc.declare_dram_parameter(
                "w" + nm, [K * K, P, Wd], cdt, isOutput=False
            )
            dram["src" + nm] = nc.declare_dram_parameter(
                "src" + nm, [SR, SW], cdt, isOutput=False
            )
            dram["h0" + nm] = nc.declare_dram_parameter(
                "h0" + nm, [P, Wd], cdt, isOutput=False
            )
            dram["out" + nm] = nc.declare_dram_parameter(
                "out" + nm, [P, Wd], cdt, isOutput=True
            )

    pool_bufs = 2 if two_byte else 1
    with tile.TileContext(nc) as tc:
        with (
            tc.tile_pool(name="srcp", bufs=pool_bufs) as srcp,
            tc.tile_pool(name="wp", bufs=3 if two_byte else 2) as wp,
            tc.tile_pool(name="pp", bufs=pool_bufs) as pp,
            tc.tile_pool(name="gp", bufs=pool_bufs) as gp,
            tc.tile_pool(name="qp", bufs=pool_bufs) as qp,
            tc.tile_pool(name="hp", bufs=pool_bufs) as hp,
            tc.tile_pool(name="accp", bufs=pool_bufs) as accp,
        ):

            def emit_body():
                for nm, P, Wd, WBLK, SR, SW in CHUNKS:
                    # ---- loads -------------------------------------------------
                    # st[p, oy, par, c] = slab[p + oy, par + c]
                    st = srcp.tile([128, K, npar, WBLK], cdt, tag="st", name="st")
                    src_in = bass.AP(
                        dram["src" + nm],
                        0,
                        [[SW, P], [SW, K], [1, npar], [1, WBLK]],
                    )
                    nc.sync.dma_start(st[0:P], src_in)

                    ht = hp.tile([128, Wd], cdt, tag="ht", name="ht")
                    nc.sync.dma_start(ht[0:P], dram["h0" + nm][:])

                    st_pstep = st.ap[0][0]  # free elems per partition

                    def win(oy, par, c0, nstep, n):
                        """source window AP: [p][tap j: step nstep][x: Wd] starting
                        at column c0 of block (oy, par)."""
                        off = (oy * npar + par) * WBLK + c0
                        return bass.AP(
                            st.tensor,
                            st.offset + off,
                            [[st_pstep, P], [nstep, n], [1, Wd]],
                        )

                    gt = gp.tile([128, K, Wd], cdt, tag="gt", name="gt")
                    for dy in range(K):
                        oy = 2 * R - dy  # slab row shift for this tap row
                        wt = wp.tile([128, K, Wd], cdt, tag="wt", name="wt")
                        w_in = bass.AP(
                            dram["w" + nm],
                            K * dy * P * Wd,
                            [[Wd, P], [P * Wd, K], [1, Wd]],
                        )
                        nc.sync.dma_start(wt[0:P], w_in)

                        pt = pp.tile([128, K, Wd], cdt, tag="pt", name="pt")
                        # ---- products: pt[:, dx, :] = w[5dy+dx] * window(ox=4-dx)
                        if npar == 2:
                            if dy != 2:
                                # dx in {0,2,4}: ox = {4,2,0}, even -> par 0
                                nc.vector.tensor_tensor(
                                    pt[0:P, 0:5:2, :],
                                    wt[0:P, 0:5:2, :],
                                    win(oy, 0, 4, -2, 3),
                                    mult,
                                )
                            else:
                                # center row: dx=2 uses h0; dx in {0,4}: ox {4,0}
                                nc.vector.tensor_tensor(
                                    pt[0:P, 0:5:4, :],
                                    wt[0:P, 0:5:4, :],
                                    win(oy, 0, 4, -4, 2),
                                    mult,
                                )
                                nc.vector.tensor_tensor(
                                    pt[0:P, 2, :], wt[0:P, 2, :], ht[0:P, :], mult
                                )
                            # dx in {1,3}: ox = {3,1}, odd -> par 1, c0 = ox-1
                            nc.vector.tensor_tensor(
                                pt[0:P, 1:4:2, :],
                                wt[0:P, 1:4:2, :],
                                win(oy, 1, 2, -2, 2),
                                mult,
                            )
                        else:
                            if dy != 2:
                                # all 5 taps in one op: dx 0..4 -> ox 4..0 step -1
                                nc.vector.tensor_tensor(
                                    pt[0:P, :, :],
                                    wt[0:P, :, :],
                                    win(oy, 0, 4, -1, K),
                                    mult,
                                )
                            else:
                                nc.vector.tensor_tensor(
                                    pt[0:P, 0:2, :],
                                    wt[0:P, 0:2, :],
                                    win(oy, 0, 4, -1, 2),
                                    mult,
                                )
                                nc.vector.tensor_tensor(
                                    pt[0:P, 3:5, :],
                                    wt[0:P, 3:5, :],
                                    win(oy, 0, 1, -1, 2),
                                    mult,
                                )
                                nc.vector.tensor_tensor(
                                    pt[0:P, 2, :], wt[0:P, 2, :], ht[0:P, :], mult
                                )
                        # ---- intra-group tree: gt[:, dy, :] = sum of 5 products
                        qt = qp.tile([128, 2, Wd], cdt, tag="qt", name="qt")
                        nc.vector.tensor_tensor(
                            qt[0:P, :, :], pt[0:P, 0:3:2, :], pt[0:P, 1:4:2, :], add
                        )
                        nc.vector.tensor_tensor(
                            pt[0:P, 0, :], qt[0:P, 0, :], qt[0:P, 1, :], add
                        )
                        nc.vector.tensor_tensor(
                            gt[0:P, dy, :], pt[0:P, 0, :], pt[0:P, 4, :], add
                        )

                    # ---- inter-group tree: out = sum of 5 group results --------
                    qf = qp.tile([128, 2, Wd], cdt, tag="qt", name="qf")
                    nc.vector.tensor_tensor(
                        qf[0:P, :, :], gt[0:P, 0:3:2, :], gt[0:P, 1:4:2, :], add
                    )
                    nc.vector.tensor_tensor(
                        gt[0:P, 0, :], qf[0:P, 0, :], qf[0:P, 1, :], add
                    )
                    at = accp.tile([128, Wd], cdt, tag="at", name="at")
                    nc.vector.tensor_tensor(
                        at[0:P, :], gt[0:P, 0, :], gt[0:P, 4, :], add
                    )
                    nc.sync.dma_start(dram["out" + nm][:], at[0:P])

            if bench:
                with tc.For_i(0, repeat, 1):
                    emit_body()
                dumt = accp.tile([1, 128], cdt, tag="dumt", name="dumt")
                nc.vector.memset(dumt[:], 0.0)
                nc.sync.dma_start(dram["_out"][:], dumt[:])
            else:
                for _rep in range(repeat):
                    emit_body()

    _split_drain_waits(nc)
    return nc


def _host_prep(guide_weight, hn, h0, np_dtype):
    """Slice and pad the full inputs into the 8 per-core input maps."""
    gw = np.asarray(guide_weight)
    hnp = np.zeros((B, H + 2 * R, W + 2 * R + 2), dtype=np_dtype)
    hnp[:, R : R + H, R : R + W] = np.asarray(hn)[:, 0]
    h0c = np.asarray(h0)[:, 0].astype(np_dtype)

    in_maps = []
    for c in range(N_CORES):
        bA, yA = c // 2, 128 * (c % 2)
        bB, colB = c // 2, 608 * (c % 2)
        wA = gw[bA, :, R + yA : R + yA + 128, R : R + W]
        wB = gw[bB, :, R + 256 : R + 352, R + colB : R + colB + 608]
        in_maps.append(
            {
                "wA": np.ascontiguousarray(wA, dtype=np_dtype),
                "srcA": np.ascontiguousarray(hnp[bA, yA : yA + 132, :]),
                "h0A": np.ascontiguousarray(h0c[bA, yA : yA + 128, :]),
                "wB": np.ascontiguousarray(wB, dtype=np_dtype),
                "srcB": np.ascontiguousarray(hnp[bB, 256:356, colB : colB + 614]),
                "h0B": np.ascontiguousarray(h0c[bB, 256:352, colB : colB + 608]),
            }
        )
    return in_maps


def _assemble(results):
    out = np.zeros((B, 1, H, W), dtype=np.float32)
    for c in range(N_CORES):
        bA, yA = c // 2, 128 * (c % 2)
        bB, colB = c // 2, 608 * (c % 2)
        out[bA, 0, yA : yA + 128, :] = results[c]["outA"].astype(np.float32)
        out[bB, 0, 256:352, colB : colB + 608] = results[c]["outB"].astype(np.float32)
    return out


_NC_CACHE = {}


def _get_nc(np_dtype, repeat=1, bench=False):
    key = (np.dtype(np_dtype).name, repeat, bench)
    if key not in _NC_CACHE:
        _NC_CACHE[key] = _build_nc(np_dtype, repeat, bench)
    return _NC_CACHE[key]


def run_on_cores(in_maps, np_dtype, repeat=1, bench=False):
    nc = _get_nc(np_dtype, repeat, bench)
    return run_bass_kernel_spmd(nc, in_maps, list(range(N_CORES)), trace=False)


def kernel(guide_weight, hn, h0, _dtype=np.float16):
    in_maps = _host_prep(guide_weight, hn, h0, np.dtype(_dtype))
    res = run_on_cores(in_maps, np.dtype(_dtype))
    return _assemble(res.results)
